# revision 8
# baseline (speedup 1.0000x reference)
"""Trainium2 Bass kernel for nn_AlternateLayer: stacked hidden-size-1 LSTMs.

Math (matching the jax reference):
  N = B*S = 2048 sequences. Per sequence: xf = flip(x, -1).reshape(T=30, 500).
  Layer 0: pre0[t] = xf[t] @ w_ih0.T + b_ih0 + b_hh0  (the only GEMM),
  then 64 stacked LSTM layers of hidden size 1 (layers 1..63 take the scalar
  h-stream of the layer below as input).

Implementation:
  - Pure data parallelism: 256 sequences per NeuronCore (8 cores).
  - Wavefront over (layer l, time t): step s processes all layers l with
    l + t = s (t = s - l), 93 steps. State H/C: [64 l, 256 n] f32; writes are
    restricted to the active layer range so inactive rows stay clean.
  - Gate preacts for ALL 64 layers come from one matmul per gate with a
    constant augmented band matrix (wih on subdiagonal, whh on diagonal,
    bias via a constant-1.0 H row, layer-0 input via pre0 rows 65:69 of H,
    refreshed per step by a PSUM->SBUF copy from the streamed GEMM).
    Output: one PSUM tile [64, 4*256] (per-gate 256-wide column slices).
  - ACT applies native Sigmoid / Tanh per gate into per-gate [64, 256] SBUF
    tiles (all partition-base 0 so the DVE update ops are start-aligned).
  - x is host-side reversed/transposed to [d=512(padded), t=30, n=256] bf16 so
    the layer-0 GEMM needs no on-chip transpose; band matmuls run as
    float32r (full PE rate at free-dim 256).
"""

import sys

sys.path.insert(0, "/opt/trn_rl_repo")

import numpy as np

import concourse.bacc as bacc
import concourse.bass as bass
import concourse.mybir as mybir
import concourse.tile as tile
from concourse.bass_utils import run_bass_kernel_spmd

B, S, T, D = 32, 64, 30, 500
L = 64
NCORES = 8
NPC = (B * S) // NCORES  # 256 sequences per core
DP = 512  # padded D
PERM = [0, 1, 3, 2]  # my gate order (i,f,o,g~) -> torch order (i,f,g,o)
NSTEPS = L + T - 1  # 93
KH = 69  # H rows: 64 state + 4 pre0 (64:68) + 1 ones (68, DMA-loaded)
BF16 = mybir.dt.np(mybir.dt.bfloat16)

_CACHE = {}


def _build_program():
    nc = bacc.Bacc(
        "TRN2",
        target_bir_lowering=False,
        debug=False,
        enable_asserts=False,
        num_devices=NCORES,
    )
    f32 = mybir.dt.float32
    f32r = mybir.dt.float32r
    bf16 = mybir.dt.bfloat16
    ACT_SIG = mybir.ActivationFunctionType.Sigmoid
    ACT_TANH = mybir.ActivationFunctionType.Tanh
    ACT_COPY = mybir.ActivationFunctionType.Copy

    xt_d = nc.dram_tensor("xt", [DP, T * NPC], bf16, kind="ExternalInput").ap()
    wg_d = nc.dram_tensor("wg", [4, 128, KH], bf16, kind="ExternalInput").ap()
    wband_d = nc.dram_tensor("wband", [KH, 4, L], bf16, kind="ExternalInput").ap()
    ones_d = nc.dram_tensor("onesrow", [1, NPC], bf16, kind="ExternalInput").ap()
    out_d = nc.dram_tensor("out", [T, NPC], bf16, kind="ExternalOutput").ap()

    with tile.TileContext(nc) as tc:
        import contextlib

        with contextlib.ExitStack() as ctx:
            consts = ctx.enter_context(tc.tile_pool(name="consts", bufs=1))
            xpool = ctx.enter_context(tc.tile_pool(name="x", bufs=1))
            state = ctx.enter_context(tc.tile_pool(name="state", bufs=1))
            ypool = ctx.enter_context(tc.tile_pool(name="y", bufs=3))
            tpool = ctx.enter_context(tc.tile_pool(name="tmp", bufs=3))
            gpool = ctx.enter_context(tc.tile_pool(name="g", bufs=2, space="PSUM"))
            ppool = ctx.enter_context(tc.tile_pool(name="p", bufs=3, space="PSUM"))

            wband = consts.tile([KH, 4, L], bf16)
            nc.sync.dma_start(wband[:], wband_d[:])
            wg = []
            for c in range(4):
                w = consts.tile([128, KH], bf16, tag=f"wg{c}", name=f"wg{c}")
                nc.sync.dma_start(w[:], wg_d[c])
                wg.append(w)

            NTG = 6
            TG = T // NTG  # 5 timesteps per DMA group
            xt = []
            for c in range(4):
                xt.append(xpool.tile([128, T * NPC], bf16, tag=f"xt{c}", name=f"xt{c}"))
            for tg in range(NTG):
                cs, ce = tg * TG * NPC, (tg + 1) * TG * NPC
                for c in range(4):
                    nc.sync.dma_start(
                        xt[c][:, cs:ce], xt_d[c * 128 : (c + 1) * 128, cs:ce]
                    )

            H = state.tile([KH, NPC], bf16)
            C = state.tile([L, NPC], f32)
            nc.vector.memset(H[:], 0.0)
            nc.vector.memset(C[:], 0.0)
            nc.sync.dma_start(H[68:69, :], ones_d[:])

            # layer-0 GEMM groups (2 timesteps each): emitted up front; the
            # ppool slots throttle them to stay a couple of steps ahead.
            NPAIR = T // 2
            pre0 = []
            for p in range(NPAIR):
                P = ppool.tile([KH, 2 * NPC], f32, tag="P", name="P")
                for c in range(4):
                    nc.tensor.matmul(
                        P[:],
                        wg[c][:],
                        xt[c][:, (2 * p) * NPC : (2 * p + 2) * NPC],
                        start=(c == 0),
                        stop=(c == 3),
                    )
                pre0.append(P)

            # --- wavefront ---
            for s in range(NSTEPS):
                lo = max(0, s - (T - 1))
                hi = min(L - 1, s)

                if s <= T - 1:
                    # refresh H pre0 rows for t = s (WAR on step s-1's matmuls)
                    P = pre0[s // 2]
                    half = (s % 2) * NPC
                    nc.scalar.activation(
                        H[64:68, :], P[64:68, half : half + NPC], ACT_COPY
                    )

                G = gpool.tile([L, 4 * NPC], f32, tag="G")
                for g in range(4):
                    nc.tensor.matmul(
                        G[:, g * NPC : (g + 1) * NPC],
                        wband[:, g, :],
                        H[:],
                        start=True,
                        stop=True,
                    )

                Yi = ypool.tile([L, NPC], f32, tag="Yi")
                Yf = ypool.tile([L, NPC], f32, tag="Yf")
                Yo = ypool.tile([L, NPC], f32, tag="Yo")
                Yg = ypool.tile([L, NPC], f32, tag="Yg")
                nc.scalar.activation(Yi[:], G[:, 0:NPC], ACT_SIG)
                nc.scalar.activation(Yf[:], G[:, NPC : 2 * NPC], ACT_SIG)
                nc.scalar.activation(Yo[:], G[:, 2 * NPC : 3 * NPC], ACT_SIG)
                nc.scalar.activation(Yg[:], G[:, 3 * NPC : 4 * NPC], ACT_TANH)

                sl = slice(32 * (lo // 32), hi + 1)
                t1 = tpool.tile([L, NPC], f32, tag="t1")
                t2 = tpool.tile([L, NPC], f32, tag="t2")
                tc_ = tpool.tile([L, NPC], f32, tag="tc")
                nc.vector.tensor_mul(t1[sl], Yf[sl], C[sl])
                nc.vector.tensor_mul(t2[sl], Yi[sl], Yg[sl])
                nc.vector.tensor_add(C[sl], t1[sl], t2[sl])
                nc.scalar.activation(tc_[sl], C[sl], ACT_TANH)
                nc.vector.tensor_mul(H[sl], Yo[sl], tc_[sl])
                if s >= L - 1:
                    t = s - (L - 1)
                    nc.sync.dma_start(out_d[t : t + 1, :], H[L - 1 : L, :])

    nc.compile()
    return nc


def _prep_core_inputs(x_shard, w_ih0, w_hh0, b_ih0, b_hh0, w_ih, w_hh, b_ih, b_hh):
    """Host-side prep of one core's input arrays."""
    xr = x_shard[:, ::-1].astype(np.float32)  # [NPC, 15000]
    xr = np.ascontiguousarray(xr).reshape(NPC, T, D)
    xp = np.zeros((NPC, T, DP), dtype=np.float32)
    xp[:, :, :D] = xr
    xt = np.ascontiguousarray(xp.transpose(2, 1, 0).reshape(DP, T * NPC))
    xt = xt.astype(BF16)

    # wg: [4][128, KH]; col 65+g gets gate g's input weights
    wpad = np.zeros((DP, 4), dtype=np.float32)
    for g in range(4):
        wpad[:D, g] = w_ih0[PERM[g], :]
    wg = np.zeros((4, 128, KH), dtype=np.float32)
    for c in range(4):
        for g in range(4):
            wg[c, :, 64 + g] = wpad[c * 128 : (c + 1) * 128, g]
    wg = wg.astype(BF16)

    # wband: [KH, 4, L]
    wband = np.zeros((KH, 4, L), dtype=np.float32)  # cast to bf16 at return
    for g in range(4):
        tg = PERM[g]
        wband[0, g, 0] = w_hh0[tg, 0]
        for l in range(1, L):
            wband[l - 1, g, l] = w_ih[l - 1, tg, 0]
            wband[l, g, l] = w_hh[l - 1, tg, 0]
        wband[68, g, 0] = b_ih0[tg] + b_hh0[tg]
        wband[68, g, 1:] = b_ih[:, tg] + b_hh[:, tg]
        wband[64 + g, g, 0] = 1.0  # pre0 delta row for layer 0
    return {
        "xt": xt,
        "wg": wg,
        "wband": wband.astype(BF16),
        "onesrow": np.ones((1, NPC), dtype=BF16),
    }


def _run(inputs, trace=False, trace_kwargs=None):
    if "nc" not in _CACHE:
        _CACHE["nc"] = _build_program()
    nc = _CACHE["nc"]

    x = np.asarray(inputs["x"], dtype=np.float32).reshape(B * S, T * D)
    params = {
        k: np.asarray(inputs[k], dtype=np.float32)
        for k in ("w_ih0", "w_hh0", "b_ih0", "b_hh0", "w_ih", "w_hh", "b_ih", "b_hh")
    }
    in_maps = []
    for i in range(NCORES):
        shard = x[i * NPC : (i + 1) * NPC]
        in_maps.append(_prep_core_inputs(shard, **params))

    res = run_bass_kernel_spmd(
        nc,
        in_maps,
        core_ids=list(range(NCORES)),
        trace=trace,
        **(trace_kwargs or {}),
    )

    out = np.empty((B * S, T), dtype=np.float32)
    for i in range(NCORES):
        out[i * NPC : (i + 1) * NPC] = np.asarray(res.results[i]["out"]).astype(np.float32).T
    return out.reshape(B, S, T), res


def kernel(**inputs):
    out, _ = _run(inputs, trace=False)
    return out
